# revision 18
# baseline (speedup 1.0000x reference)
"""Causal attention (B=4, S=4096, D_IN=768, D_OUT=64) on 8 Trainium2 NeuronCores.

Sharding: core c handles batch b=c//2 and key-parity p=c%2 (the even or odd
128-wide key tiles of that batch). Every core computes, for ALL queries of its
batch, the unnormalized attention partials over its own key set:
    num[o, q] = sum_{k in own} exp(q.k/8) * V[k, o]
    den[q]    = sum_{k in own} exp(q.k/8)
The host sums the two partials per batch and normalizes: ctx = (num/den).T.
Causality is exact: key-tile work is skipped below the diagonal band and the
two boundary blocks are masked with host-provided mask tiles.

Host prep per core: x[b].T cast to bf16, columns permuted to [own key tiles |
other key tiles] and laid out block-major ([block, partition, chunk*512]) so
each 512-column block loads with a single fully-contiguous DMA. All on-chip
matmul operands are bf16 (1 cyc/col on the PE, same rate as f32r, but half the
HBM traffic and SBUF footprint); PSUM accumulation stays fp32 and the num/den
output is written fp32. End-to-end relative error vs the fp32 reference is
~3e-3 (CPU-simulated), well inside the 2e-2 gate.

A run of zero dummy matmuls is issued at t~7us (under the initial DMA fill) so
the PE HAM clock-gate is already warm (2.4 GHz) when real work starts.
"""
import numpy as np
import ml_dtypes

import concourse.bass as bass
import concourse.bacc as bacc
import concourse.tile as tile
from concourse import mybir
from concourse.bass_utils import run_bass_kernel_spmd

B, S, DI, DO = 4, 4096, 768, 64
NCORES = 8
NIC = DI // 128          # 6 contraction chunks
NKT = S // 128           # 32 global key tiles per batch
NOWN = NKT // 2          # 16 own key tiles per core
QT = 512                 # query tile width (one PSUM bank of fp32)
NQT = S // QT            # 8 query tiles
F32 = mybir.dt.float32
BF16 = mybir.dt.bfloat16

_prog_cache = {}


def j0_of(T):
    """First diagonal-region packed key tile for permuted query tile T."""
    return 4 * T if T < 4 else 4 * (T - 4)


def build_program():
    """Build + compile the single SPMD Bass program (identical on all cores)."""
    nc = bacc.Bacc("TRN2", target_bir_lowering=False, debug=False)

    # x block-major: xb[cb][p, ic*512+w] = x[b].T[ic*128+p, perm[cb*512+w]]
    xb = nc.declare_dram_parameter("xb", [NQT, 128, NIC * QT], BF16, isOutput=False)
    # weights chunk-major: wkvb[p, ic*128+j] = [Wk|Wv][ic*128+p, j]
    wkvb = nc.declare_dram_parameter("wkvb", [128, NIC * 128], BF16, isOutput=False)
    wqb = nc.declare_dram_parameter("wqb", [128, NIC * DO], BF16, isOutput=False)
    mdiag = nc.declare_dram_parameter("mdiag", [128, 128], BF16, isOutput=False)
    mpcol = nc.declare_dram_parameter("mpcol", [128, 128], BF16, isOutput=False)
    ident = nc.declare_dram_parameter("ident", [DO, DO], BF16, isOutput=False)
    nd = nc.declare_dram_parameter("nd", [DO + 1, S], F32, isOutput=True)

    with tile.TileContext(nc) as tc:
        with tc.tile_pool(name="consts", bufs=1) as consts, \
             tc.tile_pool(name="xpool", bufs=4) as xpool, \
             tc.tile_pool(name="qkv", bufs=1) as qkv, \
             tc.tile_pool(name="expp", bufs=8) as expp, \
             tc.tile_pool(name="ndst", bufs=2) as ndst, \
             tc.tile_pool(name="ps_proj", bufs=2, space="PSUM") as ps_proj, \
             tc.tile_pool(name="ps_sc2", bufs=2, space="PSUM") as ps_sc2, \
             tc.tile_pool(name="ps_ctx", bufs=2, space="PSUM") as ps_ctx:

            # ---- PE warm-up: zero matmuls under the DMA fill so the HAM
            # clock gate reaches 8/8 before real work arrives. Nothing reads
            # the result; the run ends right around when block 0 lands.
            zb = consts.tile([128, 128], BF16, tag="zb", name="zb")
            nc.vector.memset(zb, 0.0)
            zq = consts.tile([DO, 1], F32, tag="zq", name="zq")
            nc.vector.memset(zq, 0.0)
            # Dummy exp to pull the ~2.7us ACT table load off the critical path.
            zexp = consts.tile([DO, 1], F32, tag="zexp", name="zexp")
            nc.scalar.activation(zexp, zq, mybir.ActivationFunctionType.Exp,
                                 scale=1.0)
            warmp = ps_proj.tile([128, 128], F32, tag="psproj", name="psproj")
            for _ in range(32):
                nc.tensor.matmul(warmp, zb, zb, start=True, stop=True)

            # ---- DMA issue: each DMA_DIRECT2D costs ~650ns of engine-queue
            # issue time, so the small constants go on the idle GpSimd queue
            # (parallel with Sync). Sync streams x in need order: block 0 in 6
            # chunks (first projection matmul starts on chunk 0's arrival),
            # block 1 in halves, the rest whole.
            twkv = consts.tile([128, NIC * 128], BF16, tag="twkv", name="twkv")
            twq = consts.tile([128, NIC * DO], BF16, tag="twq", name="twq")
            tmd = consts.tile([128, 128], BF16, tag="tmd", name="tmd")
            tmp = consts.tile([128, 128], BF16, tag="tmp", name="tmp")
            tid = consts.tile([DO, DO], BF16, tag="tid", name="tid")

            nc.gpsimd.dma_start(out=twkv, in_=wkvb[:, :])
            nc.gpsimd.dma_start(out=twq, in_=wqb[:, :])
            nc.gpsimd.dma_start(out=tid, in_=ident[:, :])
            nc.gpsimd.dma_start(out=tmd, in_=mdiag[:, :])
            nc.gpsimd.dma_start(out=tmp, in_=mpcol[:, :])

            # x flow control: the DMA engines round-robin bandwidth over all
            # ACTIVE queues, so an unordered issue burst starves the block
            # the PE needs first. Serialize the stream: block t+1's DMA is
            # gated on block t's arrival via a 2-element "gate" copy (reads
            # block t's tail, writes into block t+1's tile; the real DMA then
            # waits on it, WAW). Pool rotation (4 bufs) is the correctness
            # backstop against overwriting unconsumed blocks.
            xts = {}

            def load_x(cb, split, gate_on=None):
                t = xpool.tile([128, NIC * QT], BF16, tag="xt", name=f"xt_{cb}")
                xts[cb] = t
                if gate_on is not None:
                    e = NIC * QT
                    nc.vector.tensor_copy(t[0:1, 0:2], xts[gate_on][0:1, e - 2:e])
                w = NIC * QT // split
                for h in range(split):
                    nc.sync.dma_start(out=t[:, h * w:(h + 1) * w],
                                      in_=xb[cb][:, h * w:(h + 1) * w])

            load_x(0, 3)   # chunked: first projection starts on chunk 0
            load_x(1, 1, gate_on=0)
            load_x(2, 1, gate_on=1)

            def emit_next_x(st):
                if st + 3 < NQT:
                    load_x(st + 3, 1, gate_on=st + 2)

            def xc(ic, cb):
                """[128, 512] bf16 view of contraction chunk ic, column block cb."""
                return xts[cb][:, ic * QT:(ic + 1) * QT]

            # ---- projection helpers (emitted interleaved with attention) ----
            kts = [qkv.tile([DO, QT], BF16, tag=f"kt_{st}", name=f"kt_{st}") for st in range(4)]
            vts = [qkv.tile([DO, QT], BF16, tag=f"vt_{st}", name=f"vt_{st}") for st in range(4)]
            qts = [qkv.tile([DO, QT], BF16, tag=f"qt_{st}", name=f"qt_{st}") for st in range(NQT)]
            v1s = [qkv.tile([128, DO + 1], BF16, tag=f"v1_{j}", name=f"v1_{j}")
                   for j in range(NOWN)]

            def emit_pass1(st):
                """[K^T | V^T] over own key column block st, then V1 tiles."""
                p1 = ps_proj.tile([128, QT], F32, tag="psproj", name="psproj")
                for ic in range(NIC):
                    nc.tensor.matmul(p1, twkv[:, ic * 128:(ic + 1) * 128],
                                     xc(ic, st),
                                     start=(ic == 0), stop=(ic == NIC - 1))
                nc.vector.tensor_copy(kts[st], p1[0:DO, :])
                nc.vector.tensor_copy(vts[st], p1[DO:128, :])
                for j in range(4 * st, 4 * st + 4):
                    col = (j % 4) * 128
                    pv = ps_proj.tile([128, DO], BF16, tag="psproj", name="psproj")
                    nc.tensor.transpose(pv, vts[st][:, col:col + 128], tid)
                    nc.vector.tensor_copy(v1s[j][:, 0:DO], pv)
                    # ones column for the row-sum (denominator); tmd[:,127] == 1
                    nc.vector.tensor_copy(v1s[j][:, DO:DO + 1], tmd[:, 127:128])

            def emit_pass2(st):
                """Q^T over (permuted) query column block st."""
                p2 = ps_proj.tile([DO, QT], F32, tag="psproj", name="psproj")
                for ic in range(NIC):
                    nc.tensor.matmul(p2, twq[:, ic * DO:(ic + 1) * DO],
                                     xc(ic, st),
                                     start=(ic == 0), stop=(ic == NIC - 1))
                nc.vector.tensor_copy(qts[st], p2)

            # ---- attention: per query tile T, accumulate num/den over key tiles.
            # Full-width key tiles are processed in pairs sharing one 2-bank PSUM
            # tile and a single exp; the 4 diagonal-band tiles are packed 2+2.
            exp_scale = float(1.0 / np.sqrt(DO))

            def emit_scores(T, j, sp_ap):
                """scores matmul for (T, j) into sp_ap ([128, w])."""
                r = j - j0_of(T)
                qlo = 128 * r if r > 0 else 0
                w = QT - qlo
                st, col = j // 4, (j % 4) * 128
                nc.tensor.matmul(sp_ap[:, 0:w], kts[st][:, col:col + 128],
                                 qts[T][:, qlo:QT], start=True, stop=True)
                return qlo, w

            class CtxDrain:
                """Phase B for a query tile, drained a few matmuls at a time so
                ready ctx work interleaves between the next tile's scores pairs
                in the in-order PE queue."""

                def __init__(self, T, ctx_args):
                    self.T = T
                    self.nk = j0_of(T) + 4
                    self.args = ctx_args
                    self.i = 0
                    self.ctxp = ps_ctx.tile([DO + 1, QT], F32, tag="ctxp",
                                            name="ctxp")

                def drain(self, n):
                    while self.i < len(self.args) and n > 0:
                        j, et_ap, qlo, w = self.args[self.i]
                        nc.tensor.matmul(self.ctxp[:, qlo:QT], v1s[j],
                                         et_ap[:, 0:w],
                                         start=(j == 0), stop=(j == self.nk - 1))
                        self.i += 1
                        n -= 1

                def finish(self):
                    self.drain(len(self.args))
                    ost = ndst.tile([DO + 1, QT], F32, tag="ost", name="ost")
                    nc.vector.tensor_copy(ost, self.ctxp)
                    # issue the store from the (idle) gpsimd queue so output
                    # stores never delay x loads on Sync
                    nc.gpsimd.dma_start(out=nd[:, self.T * QT:(self.T + 1) * QT],
                                        in_=ost)

            # Software-pipelined emission: per query tile T we emit its scores
            # phase, then the previous tile's ctx phase (ready PE work while
            # this tile's exps run), then the projections for the NEXT column
            # block (which may stall on DMA — placed last so the in-order PE
            # queue never parks ready work behind a DMA-blocked weight load).
            emit_pass1(0)
            emit_pass2(0)
            emit_next_x(0)
            pending = None  # CtxDrain from the previous iteration
            for T in range(NQT):
                j0 = j0_of(T)
                mask = tmd if T < 4 else tmp
                ctx_args = []   # (j, et_ap, qlo, w) consumed in phase B
                for j in range(0, j0, 2):
                    sp2 = ps_sc2.tile([128, 2 * QT], F32, tag="sp2", name="sp2")
                    et2 = expp.tile([128, 2 * QT], BF16, tag="et", name="et")
                    emit_scores(T, j, sp2[:, 0:QT])
                    emit_scores(T, j + 1, sp2[:, QT:2 * QT])
                    nc.scalar.activation(et2, sp2,
                                         mybir.ActivationFunctionType.Exp,
                                         scale=exp_scale)
                    ctx_args.append((j, et2[:, 0:QT], 0, QT))
                    ctx_args.append((j + 1, et2[:, QT:2 * QT], 0, QT))
                    # interleave the previous tile's ready ctx matmuls between
                    # scores pairs: while this pair's exp frees the PSUM buf
                    # the PE chews ctx instead of blocking in-order
                    if pending is not None:
                        pending.drain(2)
                # diagonal band packed into two ps_sc2 tiles so the band never
                # touches ps_proj (which would stall the next pass1/pass2):
                # r=0 (512) + r=1 (384) + r=3 (128) fill one 2-bank tile
                # exactly; r=2 (256) goes in a second.
                spb1 = ps_sc2.tile([128, 2 * QT], F32, tag="sp2", name="sp2")
                etb1 = expp.tile([128, 2 * QT], BF16, tag="et", name="et")
                emit_scores(T, j0, spb1[:, 0:QT])
                emit_scores(T, j0 + 1, spb1[:, QT:QT + 384])
                emit_scores(T, j0 + 3, spb1[:, QT + 384:2 * QT])
                nc.scalar.activation(etb1, spb1,
                                     mybir.ActivationFunctionType.Exp,
                                     scale=exp_scale)
                nc.vector.tensor_mul(etb1[:, 0:128], etb1[:, 0:128], mask)
                nc.vector.tensor_mul(etb1[:, QT:QT + 128], etb1[:, QT:QT + 128], mask)
                nc.vector.tensor_mul(etb1[:, QT + 384:2 * QT],
                                     etb1[:, QT + 384:2 * QT], mask)
                if pending is not None:
                    pending.drain(3)
                spb2 = ps_sc2.tile([128, 2 * QT], F32, tag="sp2", name="sp2")
                etb2 = expp.tile([128, 2 * QT], BF16, tag="et", name="et")
                emit_scores(T, j0 + 2, spb2[:, 0:256])
                nc.scalar.activation(etb2[:, 0:256], spb2[:, 0:256],
                                     mybir.ActivationFunctionType.Exp,
                                     scale=exp_scale)
                nc.vector.tensor_mul(etb2[:, 0:128], etb2[:, 0:128], mask)
                ctx_args.append((j0, etb1[:, 0:QT], 0, QT))
                ctx_args.append((j0 + 1, etb1[:, QT:QT + 384], 128, 384))
                ctx_args.append((j0 + 2, etb2[:, 0:256], 256, 256))
                ctx_args.append((j0 + 3, etb1[:, QT + 384:2 * QT], 384, 128))

                if pending is not None:
                    pending.finish()
                pending = CtxDrain(T, ctx_args)
                if T + 1 < NQT:
                    if T + 1 < 4:
                        emit_pass1(T + 1)
                    emit_pass2(T + 1)
                    emit_next_x(T + 1)
            pending.finish()

    nc.compile()
    return nc


def get_program():
    if "nc" not in _prog_cache:
        _prog_cache["nc"] = build_program()
    return _prog_cache["nc"]


def core_perm(parity):
    """Permuted-to-global column index map: own key tiles first, then other."""
    own = [g for g in range(NKT) if g % 2 == parity]
    other = [g for g in range(NKT) if g % 2 != parity]
    return np.concatenate([np.arange(g * 128, (g + 1) * 128) for g in own + other])


def make_in_maps(x, Wq, Wk, Wv):
    x = np.asarray(x, dtype=np.float32)
    Wq = np.asarray(Wq, dtype=np.float32)
    Wk = np.asarray(Wk, dtype=np.float32)
    Wv = np.asarray(Wv, dtype=np.float32)
    bf = ml_dtypes.bfloat16
    wkv = np.concatenate([Wk, Wv], axis=1)          # [768, 128]
    # chunk-major: [p, ic*128+j]
    wkvb = np.ascontiguousarray(
        wkv.reshape(NIC, 128, 128).transpose(1, 0, 2).reshape(128, NIC * 128)
    ).astype(bf)
    wqb = np.ascontiguousarray(
        Wq.reshape(NIC, 128, DO).transpose(1, 0, 2).reshape(128, NIC * DO)
    ).astype(bf)
    mdiag = np.triu(np.ones((128, 128), dtype=np.float32)).astype(bf)
    ident = np.eye(DO, dtype=np.float32).astype(bf)
    in_maps = []
    perms = []
    for c in range(NCORES):
        b, par = c // 2, c % 2
        perm = core_perm(par)
        perms.append(perm)
        xTp = x[b].T[:, perm].astype(bf)            # [768, 4096] bf16
        # block-major: xb[cb][p, ic*512+w] = xTp[ic*128+p, cb*512+w]
        xbm = np.ascontiguousarray(
            xTp.reshape(NIC, 128, NQT, QT).transpose(2, 1, 0, 3)
               .reshape(NQT, 128, NIC * QT))
        mpcol = np.full((128, 128), 1.0 - par, dtype=np.float32).astype(bf)
        in_maps.append({
            "xb": xbm, "wkvb": wkvb, "wqb": wqb,
            "mdiag": mdiag, "mpcol": mpcol, "ident": ident,
        })
    return in_maps, perms


def combine(results, perms):
    out = np.empty((B, S, DO), dtype=np.float32)
    for b in range(B):
        num = np.zeros((DO, S), dtype=np.float64)
        den = np.zeros((S,), dtype=np.float64)
        for c in (2 * b, 2 * b + 1):
            nd_c = results[c]["nd"].astype(np.float64)
            inv = np.empty(S, dtype=np.int64)
            inv[perms[c]] = np.arange(S)
            nd_g = nd_c[:, inv]
            num += nd_g[:DO]
            den += nd_g[DO]
        out[b] = (num / den).T.astype(np.float32)
    return out


def kernel(x, Wq, Wk, Wv):
    nc = get_program()
    in_maps, perms = make_in_maps(x, Wq, Wk, Wv)
    res = run_bass_kernel_spmd(nc, in_maps, list(range(NCORES)))
    return combine(res.results, perms)
